# revision 3
# baseline (speedup 1.0000x reference)
"""Trainium2 Bass kernel for nn_DKAModule (dynamic-kernel attention), v2.

Data-parallel over B*n = 8192 tokens -> TPC=1024 per core (+10-token halo).
All matmuls in bf16 (1 cycle/col on PE, f32 PSUM accumulation).

Per core:
  xp^T   = W_in^T-blocks @ x^T blocks             (PE, 8 m-groups x 3 chunks)
  per head h (d_h = 128 partitions, window k_h, rank R=4):
    xtd  = tiled transpose of xp_h                (1 DMA-transpose, 10 tiles)
    S    = banded conv matmuls per 128-token tile (PE: center + L/R halo)
    cs   = S * c broadcast                        (DVE fused PSUM evac, or
                                                   Act copy + Pool mult)
    o_h  = sum_r V'_r (x) cs_r + sum_j g_j (x) shift_j(xp_h)
                                                  (DVE tensor_scalar MAC
                                                   chains, Pool combines)
  out    = o^T-blocks @ W_out^T + b_out           (PE, PSUM -> DRAM DMA)

c = x_proj_head @ Wc is computed on host (folded into x @ Wc_aug) and
broadcast across partitions by a stride-0 DMA read of compact c rows.
"""
import sys
import types

import ml_dtypes
import numpy as np

BF16 = ml_dtypes.bfloat16

KS = [3, 3, 7, 7, 11, 11, 21, 21]
H, DM, DH, R, B, N = 8, 1024, 128, 4, 2, 4096
NC = 8
TPC = B * N // NC  # tokens per core
PAD = 10  # halo tokens each side
LP = 128  # left zero-pad columns in xp
XF = 1280  # padded xp width = 10 transpose tiles
NT = TPC // 128  # 8 output tiles
TH = TPC + 2 * PAD  # 1044 valid x columns
HEADS = (4, 0, 5, 1, 6, 2, 7, 3)  # s1 m-group / head processing order
# heads whose cs evac goes Act-copy + Pool-mult instead of fused DVE
PATHB_HEADS = (0, 1, 2)
S1CH = [(0, 348), (348, 348), (696, 348)]

_MODULE_CACHE = {}


def _install_ntff_hook_shim():
    """This image's antenv lacks axon_hooks; provide it so profiling works."""
    if "antenv.axon_hooks" in sys.modules:
        return
    try:
        from trn_agent_boot.trn_boot import _ntff_profile_via_ctypes

        hook = _ntff_profile_via_ctypes("/opt/axon/libaxon_pjrt.so")
    except Exception:
        hook = None
    mod = types.ModuleType("antenv.axon_hooks")
    mod.get_axon_ntff_profile_hook = lambda: hook
    mod.set_axon_ntff_profile_hook = lambda h: None
    sys.modules["antenv.axon_hooks"] = mod


def _split_multi_waits(nc, mybir):
    """walrus codegen allows a single sync-wait per instruction; hoist
    extras onto a chain of single-wait NoOps on the same engine."""
    for f in nc.m.functions:
        for blk in f.blocks:
            new_insts = []
            for inst in blk.instructions:
                si = getattr(inst, "sync_info", None)
                ow = list(si.on_wait) if si and si.on_wait else []
                if len(ow) >= 2:
                    for i, w in enumerate(ow[:-1]):
                        new_insts.append(
                            mybir.InstNoOp(
                                name=f"{inst.name}-wn{i}",
                                ins=[],
                                outs=[],
                                engine=inst.engine,
                                sync_info=mybir.SyncInfo(on_wait=[w], on_update=[]),
                            )
                        )
                    inst.sync_info = mybir.SyncInfo(
                        on_wait=[ow[-1]],
                        on_update=list(si.on_update) if si.on_update else [],
                    )
                new_insts.append(inst)
            blk.instructions = new_insts


def _band_off(h):
    """Column offset of head h's packed band block and its width."""
    off = 0
    for g in range(h):
        p = KS[g] // 2
        off += R * (128 + 2 * p)
    p = KS[h] // 2
    return off, R * (128 + 2 * p)


def _build_module(has_bias):
    import concourse.bass as bass
    import concourse.tile as tile
    from concourse import mybir

    f32 = mybir.dt.float32
    bf16 = mybir.dt.bfloat16
    MULT = mybir.AluOpType.mult
    ADD = mybir.AluOpType.add
    IDENT = mybir.ActivationFunctionType.Identity

    nc = bass.Bass(trn_type="TRN2")

    # ---- DRAM I/O ----
    xT_d = nc.dram_tensor("xT", [DM, TH], bf16, kind="ExternalInput")
    w_inT_d = nc.dram_tensor("w_inT", [DM, DM], bf16, kind="ExternalInput")
    w_outT_d = nc.dram_tensor("w_outT", [DM, DM], bf16, kind="ExternalInput")
    c_d = nc.dram_tensor("c", [H * R, TPC], bf16, kind="ExternalInput")
    band_total = _band_off(H - 1)[0] + _band_off(H - 1)[1]
    band_d = nc.dram_tensor("band", [128, band_total], bf16, kind="ExternalInput")
    gvec_d = nc.dram_tensor("gvec", [DH, H * 21], f32, kind="ExternalInput")
    vcol_d = nc.dram_tensor("vcol", [DH, H * R], f32, kind="ExternalInput")
    b_in_d = nc.dram_tensor("b_in", [DM, 1], f32, kind="ExternalInput")
    if has_bias:
        b_out_d = nc.dram_tensor("b_out", [1, DM], bf16, kind="ExternalInput")
    out_d = nc.dram_tensor("out", [TPC, DM], f32, kind="ExternalOutput")

    with tile.TileContext(nc) as tc:
        with tc.tile_pool(name="const", bufs=1) as pc:
            xp_sb = [pc.tile([DH, XF], bf16, name=f"xp{m}") for m in range(H)]
            o_sb = [pc.tile([DH, TPC], bf16, name=f"o{h}") for h in range(H)]
            w_sb = [pc.tile([128, DM], bf16, name=f"w_in{i}") for i in range(H)]
            xT_sb = [pc.tile([128, TH], bf16, name=f"xT{i}") for i in range(H)]
            wo_sb = [pc.tile([128, DM], bf16, name=f"w_out{i}") for i in range(H)]
            gvec_sb = pc.tile([DH, H * 21], f32, name="gvec_sb")
            vcol_sb = pc.tile([DH, H * R], f32, name="vcol_sb")
            b_in_sb = pc.tile([128, H], f32, name="b_in_sb")
            if has_bias:
                ones_sb = pc.tile([1, 128], bf16, name="ones_sb")
                bo_sb = pc.tile([1, DM], bf16, name="bo_sb")
                nc.gpsimd.memset(ones_sb, 1.0)
                nc.sync.dma_start(out=bo_sb, in_=b_out_d[:, :])

            # zero the transpose padding regions of xp
            for m in range(H):
                nc.gpsimd.memset(xp_sb[m][:, 0 : LP - PAD], 0)
                nc.gpsimd.memset(xp_sb[m][:, LP + TPC + PAD : XF], 0)

            # ---- preamble DMAs (SP engine), highest priority first ----
            m0 = HEADS[0]
            for i in range(H):
                nc.sync.dma_start(
                    out=w_sb[i][:, m0 * 128 : (m0 + 1) * 128],
                    in_=w_inT_d[i * 128 : (i + 1) * 128, m0 * 128 : (m0 + 1) * 128],
                )
            for i in range(H):
                nc.sync.dma_start(
                    out=xT_sb[i][:, 0:348], in_=xT_d[i * 128 : (i + 1) * 128, 0:348]
                )
            nc.sync.dma_start(out=gvec_sb, in_=gvec_d[:, :])
            nc.sync.dma_start(out=vcol_sb, in_=vcol_d[:, :])
            for m in range(H):
                nc.sync.dma_start(
                    out=b_in_sb[:, m : m + 1], in_=b_in_d[m * 128 : (m + 1) * 128, :]
                )
            for i in range(H):
                nc.sync.dma_start(
                    out=xT_sb[i][:, 348:TH], in_=xT_d[i * 128 : (i + 1) * 128, 348:TH]
                )
            # rest of w_in, skipping the already-loaded m0 block
            for i in range(H):
                nc.sync.dma_start(
                    out=w_sb[i][:, 0 : m0 * 128],
                    in_=w_inT_d[i * 128 : (i + 1) * 128, 0 : m0 * 128],
                )
                nc.sync.dma_start(
                    out=w_sb[i][:, (m0 + 1) * 128 : DM],
                    in_=w_inT_d[i * 128 : (i + 1) * 128, (m0 + 1) * 128 : DM],
                )

            with tc.tile_pool(name="ps1", bufs=2, space="PSUM") as pp1, tc.tile_pool(
                name="ps3", bufs=3, space="PSUM"
            ) as pp3, tc.tile_pool(name="pband", bufs=3) as p_band, tc.tile_pool(
                name="pcb", bufs=2
            ) as p_cb, tc.tile_pool(name="pxtd", bufs=3) as p_xtd, tc.tile_pool(
                name="pchain", bufs=2
            ) as p_ch, tc.tile_pool(name="psS", bufs=3) as p_sS:
                band_tiles = {}
                cb_tiles = {}
                xtd_tiles = {}

                def issue_head_dmas(h):
                    boff, bw = _band_off(h)
                    bt = p_band.tile([128, bw], bf16, name=f"band{h}", tag="band")
                    nc.sync.dma_start(out=bt, in_=band_d[:, boff : boff + bw])
                    band_tiles[h] = bt
                    cb = p_cb.tile([128, R * TPC], bf16, name=f"cb{h}", tag="cb")
                    nc.sync.dma_start(
                        out=cb,
                        in_=c_d[R * h : R * (h + 1), :].partition_broadcast(128),
                    )
                    cb_tiles[h] = cb

                def stage3(h):
                    k = KS[h]
                    p = k // 2
                    bt = band_tiles.pop(h)
                    bC = bt[:, 0 : R * 128].rearrange("q (r w) -> q r w", r=R)
                    bL = bt[:, R * 128 : R * 128 + R * p].rearrange(
                        "q (r w) -> q r w", r=R
                    )
                    bR = bt[:, R * 128 + R * p : R * 128 + 2 * R * p].rearrange(
                        "q (r w) -> q r w", r=R
                    )
                    cb = cb_tiles.pop(h)
                    cb3 = cb.rearrange("q (r t) -> q r t", r=R)
                    xtd = xtd_tiles.pop(h)
                    pathb = h in PATHB_HEADS
                    for b in range(NT):
                        ps_s = pp3.tile([128, R, 128], f32, name="ps_s", tag="ps_s")
                        nc.tensor.matmul(
                            ps_s, xtd[:, b + 1, :], bC, start=True, stop=False
                        )
                        nc.tensor.matmul(
                            ps_s[:, :, 0:p], xtd[:, b, :], bL, start=False, stop=False
                        )
                        nc.tensor.matmul(
                            ps_s[:, :, 128 - p : 128],
                            xtd[:, b + 2, :],
                            bR,
                            start=False,
                            stop=True,
                        )
                        csl = cb3[:, :, b * 128 : (b + 1) * 128]
                        if pathb:
                            st = p_sS.tile([128, R, 128], bf16, name="sS", tag="sS")
                            nc.scalar.copy(st, ps_s)
                            nc.gpsimd.tensor_mul(csl, st, csl)
                        else:
                            nc.vector.tensor_mul(csl, ps_s, csl)

                    # ---- MAC chains (DVE) ----
                    gv = gvec_sb
                    vc = vcol_sb

                    def chain(tile_out, taps, first_ts):
                        """taps: list of (in0_ap, scalar_ap); first via
                        tensor_scalar, rest via STT accumulate."""
                        in0, sc = taps[0]
                        nc.vector.tensor_scalar(
                            out=tile_out, in0=in0, scalar1=sc, scalar2=None, op0=MULT
                        )
                        for in0, sc in taps[1:]:
                            nc.vector.scalar_tensor_tensor(
                                out=tile_out,
                                in0=in0,
                                scalar=sc,
                                in1=tile_out,
                                op0=MULT,
                                op1=ADD,
                            )

                    dyn_taps = [
                        (cb3[:, r, :], vc[:, h * R + r : h * R + r + 1])
                        for r in range(R)
                    ]
                    stat_taps = [
                        (
                            xp_sb[h][:, LP + j - p : LP + j - p + TPC],
                            gv[:, h * 21 + j : h * 21 + j + 1],
                        )
                        for j in range(k)
                    ]
                    if k == 3:
                        # single chain of 7, last STT writes o_sb directly
                        taps = dyn_taps + stat_taps
                        acc = p_ch.tile([DH, TPC], bf16, name="acc", tag="acc")
                        chain(acc, taps[:-1], True)
                        in0, sc = taps[-1]
                        nc.vector.scalar_tensor_tensor(
                            out=o_sb[h], in0=in0, scalar=sc, in1=acc, op0=MULT, op1=ADD
                        )
                    elif k == 7:
                        acc = p_ch.tile([DH, TPC], bf16, name="acc", tag="acc")
                        c2 = p_ch.tile([DH, TPC], bf16, name="c2", tag="c2")
                        chain(acc, dyn_taps, True)
                        chain(c2, stat_taps, True)
                        nc.gpsimd.tensor_add(o_sb[h], acc, c2)
                    elif k == 11:
                        acc = p_ch.tile([DH, TPC], bf16, name="acc", tag="acc")
                        c2 = p_ch.tile([DH, TPC], bf16, name="c2", tag="c2")
                        c3 = p_ch.tile([DH, TPC], bf16, name="c3", tag="c3")
                        chain(acc, dyn_taps, True)
                        chain(c2, stat_taps[:6], True)
                        chain(c3, stat_taps[6:], True)
                        nc.gpsimd.tensor_add(c2, c2, c3)
                        nc.gpsimd.tensor_add(o_sb[h], acc, c2)
                    else:  # k == 21
                        acc = p_ch.tile([DH, TPC], bf16, name="acc", tag="acc")
                        c2 = p_ch.tile([DH, TPC], bf16, name="c2", tag="c2")
                        c3 = p_ch.tile([DH, TPC], bf16, name="c3", tag="c3")
                        c4 = p_ch.tile([DH, TPC], bf16, name="c4", tag="c4")
                        chain(acc, dyn_taps, True)
                        chain(c2, stat_taps[0:7], True)
                        chain(c3, stat_taps[7:14], True)
                        chain(c4, stat_taps[14:21], True)
                        nc.gpsimd.tensor_add(c2, c2, c3)
                        nc.gpsimd.tensor_add(acc, acc, c4)
                        nc.gpsimd.tensor_add(o_sb[h], acc, c2)

                # ---------------- stage 1 + stage 3 pipeline ----------------
                for gi, m in enumerate(HEADS):
                    issue_head_dmas(m)
                    if gi == 2:
                        for i in range(H):
                            nc.sync.dma_start(
                                out=wo_sb[i], in_=w_outT_d[i * 128 : (i + 1) * 128, :]
                            )
                    for c0, cn in S1CH:
                        ps1 = pp1.tile([128, 348], f32, name="ps1", tag="ps1")
                        for i in range(H):
                            nc.tensor.matmul(
                                ps1[:, :cn],
                                w_sb[i][:, m * 128 : (m + 1) * 128],
                                xT_sb[i][:, c0 : c0 + cn],
                                start=(i == 0),
                                stop=(i == H - 1),
                            )
                        nc.scalar.activation(
                            out=xp_sb[m][:, LP - PAD + c0 : LP - PAD + c0 + cn],
                            in_=ps1[:, :cn],
                            func=IDENT,
                            bias=b_in_sb[:, m : m + 1],
                            scale=1.0,
                        )
                    xtd = p_xtd.tile([128, XF // 128, 128], bf16, name="xtd", tag="xtd")
                    nc.scalar.dma_start_transpose(out=xtd, in_=xp_sb[m])
                    xtd_tiles[m] = xtd
                    if gi >= 1:
                        stage3(HEADS[gi - 1])
                stage3(HEADS[-1])

            # ---------------- stage 4: out projection ----------------
            with tc.tile_pool(name="ps4", bufs=4, space="PSUM") as pp4, tc.tile_pool(
                name="post", bufs=4
            ) as p_ost:
                for t in range(NT):
                    for ei, e0 in enumerate((0, 512)):
                        ps4 = pp4.tile([128, 512], f32, name="ps4", tag="ps4")
                        n_mm = H + (1 if has_bias else 0)
                        for i in range(H):
                            nc.tensor.matmul(
                                ps4,
                                o_sb[i][:, t * 128 : (t + 1) * 128],
                                wo_sb[i][:, e0 : e0 + 512],
                                start=(i == 0),
                                stop=(i == n_mm - 1),
                            )
                        if has_bias:
                            nc.tensor.matmul(
                                ps4,
                                ones_sb,
                                bo_sb[:, e0 : e0 + 512],
                                start=False,
                                stop=True,
                            )
                        ost = p_ost.tile([128, 512], f32, name="ost", tag="ost")
                        if ei == 0:
                            nc.scalar.copy(ost, ps4)
                        else:
                            nc.vector.tensor_scalar(
                                out=ost, in0=ps4, scalar1=1.0, scalar2=None, op0=MULT
                            )
                        eng = nc.sync if ei == 0 else nc.scalar
                        eng.dma_start(
                            out=out_d[t * 128 : (t + 1) * 128, e0 : e0 + 512],
                            in_=ost,
                        )

    _split_multi_waits(nc, mybir)
    return nc


def _host_prep(inputs):
    x = np.ascontiguousarray(np.asarray(inputs["x"], dtype=np.float32))
    W_in = np.asarray(inputs["W_in"], dtype=np.float32)
    b_in = np.asarray(inputs["b_in"], dtype=np.float32)
    W_out = np.asarray(inputs["W_out"], dtype=np.float32)
    b_out = np.asarray(inputs["b_out"], dtype=np.float32)
    Wc = np.asarray(inputs["Wc"], dtype=np.float32)
    A = np.asarray(inputs["A"], dtype=np.float32)
    V = np.asarray(inputs["V"], dtype=np.float32)
    base = np.asarray(inputs["base"], dtype=np.float32)
    alphas = np.asarray(inputs["alphas"], dtype=np.float32)

    alpha = 1.0 / (1.0 + np.exp(-alphas))
    W_inT = np.ascontiguousarray(W_in.T)
    W_outT = np.ascontiguousarray(W_out.T)
    Wc_aug = np.zeros((DM, H * R), dtype=np.float32)
    for h in range(H):
        Wc_aug[:, R * h : R * h + R] = W_inT[:, h * DH : (h + 1) * DH] @ Wc[h]

    # packed per-head band blocks: [C (R,128) | L (R,p) | R (R,p)]
    band_total = _band_off(H - 1)[0] + _band_off(H - 1)[1]
    band = np.zeros((128, band_total), dtype=np.float32)
    for h in range(H):
        k = KS[h]
        p = k // 2
        boff, _ = _band_off(h)
        t = np.arange(128)[:, None]
        w = np.arange(128)[None, :]
        # C: A[r, t-w+p] where |t-w| <= p
        d = t - w
        mC = np.abs(d) <= p
        dC = np.clip(d + p, 0, k - 1)
        # L: w in [0,p): A[r, t-w-128+p] for t >= 128+w-p
        wl = np.arange(p)[None, :]
        dL = t - wl - 128
        mL = (dL >= -p) & (dL <= p)
        dLc = np.clip(dL + p, 0, k - 1)
        # R: u in [0,p) (w = 128-p+u): A[r, t-u+2p...] j = t-(u-p)-128+128?
        u = np.arange(p)[None, :]
        dR = t + 128 - (128 - p + u)  # t - w + 128
        mR = (dR >= -p) & (dR <= p)
        dRc = np.clip(dR + p, 0, k - 1)
        for r in range(R):
            Ar = A[h, r]
            band[:, boff + r * 128 : boff + (r + 1) * 128] = np.where(mC, Ar[dC], 0.0)
            lo = boff + R * 128
            band[:, lo + r * p : lo + (r + 1) * p] = np.where(mL, Ar[dLc], 0.0)
            lo = boff + R * 128 + R * p
            band[:, lo + r * p : lo + (r + 1) * p] = np.where(mR, Ar[dRc], 0.0)

    gvec = np.zeros((DH, H, 21), dtype=np.float32)
    vcol = np.zeros((DH, H, R), dtype=np.float32)
    for h in range(H):
        k = KS[h]
        gvec[:, h, :k] = ((1.0 - alpha[h]) * base[h, :k]).T
        vcol[:, h, :] = (alpha[h] * V[h]).T

    prep = {
        "w_inT": W_inT.astype(BF16),
        "w_outT": W_outT.astype(BF16),
        "band": band.astype(BF16),
        "gvec": gvec.reshape(DH, H * 21).copy(),
        "vcol": vcol.reshape(DH, H * R).copy(),
        "b_in": b_in.reshape(DM, 1).copy(),
    }
    has_bias = bool(np.any(b_out != 0.0))
    if has_bias:
        prep["b_out"] = b_out.reshape(1, DM).astype(BF16)

    xT_slices = []
    c_slices = []
    per_b = NC // B
    for c in range(NC):
        bb = c // per_b
        s = (c % per_b) * TPC
        sl = np.zeros((TH, DM), dtype=np.float32)
        lo, hi = s - PAD, s + TPC + PAD
        clo, chi = max(lo, 0), min(hi, N)
        sl[clo - lo : chi - lo] = x[bb, clo:chi]
        xT_slices.append(np.ascontiguousarray(sl.T).astype(BF16))
        cc = sl[PAD : PAD + TPC] @ Wc_aug  # (TPC, H*R)
        c_slices.append(np.ascontiguousarray(cc.T).astype(BF16))
    return prep, xT_slices, c_slices, has_bias


def _run(inputs, trace=False, **kwargs):
    _install_ntff_hook_shim()
    from concourse.bass_utils import run_bass_kernel_spmd

    prep, xT_slices, c_slices, has_bias = _host_prep(inputs)
    key = ("mod", has_bias)
    if key not in _MODULE_CACHE:
        _MODULE_CACHE[key] = _build_module(has_bias)
    nc = _MODULE_CACHE[key]

    in_maps = []
    for c in range(NC):
        m = dict(prep)
        m["xT"] = xT_slices[c]
        m["c"] = c_slices[c]
        in_maps.append(m)

    res = run_bass_kernel_spmd(
        nc, in_maps, core_ids=list(range(NC)), trace=trace, **kwargs
    )
    outs = [res.results[c]["out"] for c in range(NC)]
    full = np.concatenate(outs, axis=0).reshape(B, N, DM).astype(np.float32)
    return full, res


def kernel(**inputs) -> np.ndarray:
    return _run(inputs)[0]


# revision 4
# speedup vs baseline: 1.5150x; 1.5150x over previous
"""Trainium2 Bass kernel for nn_DKAModule (dynamic-kernel attention), v3.

Data-parallel over B*n = 8192 tokens -> TPC=1024 per core (+10-token halo).
All matmuls bf16 (1 cycle/col on PE at 2.4GHz when back-to-back).

Per core, software-pipelined over heads (group g runs stage1 of head g,
band matmuls of head g-1, diag/chain tail of head g-2):

  stage1:  xp_m = W_in-block^T @ x^T + b_in     (PE; Act evac to bf16)
  band:    xtd  = 10-tile DMA transpose of xp_h (1 trigger, SP)
           ps_s = per-128-token-tile banded-conv matmuls (PE: C + L/R
                  halo slices, accumulated in PSUM). The per-token
                  coefficients c (= alpha * x_proj_h @ Wc) are folded
                  into the band matrices ON HOST, so ps_s = S*c already.
           cs   = plain PSUM evac (Act copy, bf16)
  tail:    o_h  = sum_r diag(V_r) @ cs_r        (PE, vdiag)
                + static conv:  k=11 heads via diag matmuls (PE, gdiag)
                                k=3,7,21 heads via DVE STT MAC chains
                  (chains <= 7 long for bf16 accumulation error)
  stage4:  out  = o^T-blocks @ W_out^T (+ b_out) (PE; Act/DVE evac)
"""
import sys
import types

import ml_dtypes
import numpy as np

BF16 = ml_dtypes.bfloat16

KS = [3, 3, 7, 7, 11, 11, 21, 21]
H, DM, DH, R, B, N = 8, 1024, 128, 4, 2, 4096
NC = 8
TPC = B * N // NC
PAD = 10
LP = 128  # left zero-pad columns in xp
XF = 1280  # padded xp width = 10 transpose tiles
NT = TPC // 128  # 8 token tiles
TH = TPC + 2 * PAD  # 1044 valid x columns
HEADS = (6, 7, 4, 5, 2, 3, 1, 0)  # k=21 first (long DVE chains), k=3 last
PE_STATIC_HEADS = (4, 5)  # k=11 static conv via PE diag matmuls
S1CH = [(0, 512), (512, 512), (1024, 20)]

_MODULE_CACHE = {}


def _install_ntff_hook_shim():
    """This image's antenv lacks axon_hooks; provide it so profiling works."""
    if "antenv.axon_hooks" in sys.modules:
        return
    try:
        from trn_agent_boot.trn_boot import _ntff_profile_via_ctypes

        hook = _ntff_profile_via_ctypes("/opt/axon/libaxon_pjrt.so")
    except Exception:
        hook = None
    mod = types.ModuleType("antenv.axon_hooks")
    mod.get_axon_ntff_profile_hook = lambda: hook
    mod.set_axon_ntff_profile_hook = lambda h: None
    sys.modules["antenv.axon_hooks"] = mod


def _split_multi_waits(nc, mybir):
    """walrus codegen allows a single sync-wait per instruction; hoist
    extras onto a chain of single-wait NoOps on the same engine."""
    for f in nc.m.functions:
        for blk in f.blocks:
            new_insts = []
            for inst in blk.instructions:
                si = getattr(inst, "sync_info", None)
                ow = list(si.on_wait) if si and si.on_wait else []
                if len(ow) >= 2:
                    for i, w in enumerate(ow[:-1]):
                        new_insts.append(
                            mybir.InstNoOp(
                                name=f"{inst.name}-wn{i}",
                                ins=[],
                                outs=[],
                                engine=inst.engine,
                                sync_info=mybir.SyncInfo(on_wait=[w], on_update=[]),
                            )
                        )
                    inst.sync_info = mybir.SyncInfo(
                        on_wait=[ow[-1]],
                        on_update=list(si.on_update) if si.on_update else [],
                    )
                new_insts.append(inst)
            blk.instructions = new_insts


def _tile_cols(h):
    """Band cols per token-tile for head h: C (R*128) + L (R*p) + R (R*p)."""
    p = KS[h] // 2
    return R * (128 + 2 * p)


def _band_off(h):
    """Column offset of head h's packed per-tile band blocks."""
    off = 0
    for g in range(h):
        off += NT * _tile_cols(g)
    return off


BAND_TOTAL = _band_off(H - 1) + NT * _tile_cols(H - 1)

# gdiag packing (PE-static heads only)
GD_OFF = {}
_o = 0
for _h in PE_STATIC_HEADS:
    GD_OFF[_h] = _o
    _o += KS[_h] * DH
GD_TOTAL = _o


def _build_module(has_bias):
    import concourse.bass as bass
    import concourse.tile as tile
    from concourse import mybir

    f32 = mybir.dt.float32
    bf16 = mybir.dt.bfloat16
    MULT = mybir.AluOpType.mult
    ADD = mybir.AluOpType.add
    IDENT = mybir.ActivationFunctionType.Identity

    nc = bass.Bass(trn_type="TRN2")

    xT_d = nc.dram_tensor("xT", [DM, TH], bf16, kind="ExternalInput")
    w_inT_d = nc.dram_tensor("w_inT", [DM, DM], bf16, kind="ExternalInput")
    w_outT_d = nc.dram_tensor("w_outT", [DM, DM], bf16, kind="ExternalInput")
    band_d = nc.dram_tensor("band", [128, BAND_TOTAL], bf16, kind="ExternalInput")
    vdiag_d = nc.dram_tensor("vdiag", [DH, H * R * DH], bf16, kind="ExternalInput")
    gdiag_d = nc.dram_tensor("gdiag", [DH, GD_TOTAL], bf16, kind="ExternalInput")
    gvec_d = nc.dram_tensor("gvec", [DH, H * 21], f32, kind="ExternalInput")
    b_in_d = nc.dram_tensor("b_in", [DM, 1], f32, kind="ExternalInput")
    if has_bias:
        b_out_d = nc.dram_tensor("b_out", [1, DM], bf16, kind="ExternalInput")
    out_d = nc.dram_tensor("out", [TPC, DM], f32, kind="ExternalOutput")

    with tile.TileContext(nc) as tc:
        with tc.tile_pool(name="const", bufs=1) as pc:
            xp_sb = [pc.tile([DH, XF], bf16, name=f"xp{m}") for m in range(H)]
            o_sb = [pc.tile([DH, TPC], bf16, name=f"o{h}") for h in range(H)]
            w_sb = [pc.tile([128, DM], bf16, name=f"w_in{i}") for i in range(H)]
            xT_sb = [pc.tile([128, TH], bf16, name=f"xT{i}") for i in range(H)]
            wo_sb = [pc.tile([128, DM], bf16, name=f"w_out{i}") for i in range(H)]
            gvec_sb = pc.tile([DH, H * 21], f32, name="gvec_sb")
            vd_sb = pc.tile([DH, H * R * DH], bf16, name="vd_sb")
            gd_sb = pc.tile([DH, GD_TOTAL], bf16, name="gd_sb")
            b_in_sb = pc.tile([128, H], f32, name="b_in_sb")
            if has_bias:
                ones_sb = pc.tile([1, 128], bf16, name="ones_sb")
                bo_sb = pc.tile([1, DM], bf16, name="bo_sb")
                nc.gpsimd.memset(ones_sb, 1.0)
                nc.sync.dma_start(out=bo_sb, in_=b_out_d[:, :])

            for m in range(H):
                nc.gpsimd.memset(xp_sb[m][:, 0 : LP - PAD], 0)
                nc.gpsimd.memset(xp_sb[m][:, LP + TPC + PAD : XF], 0)

            # ---- preamble DMAs (SP), first-needed first ----
            m0 = HEADS[0]
            for i in range(H):
                nc.sync.dma_start(
                    out=w_sb[i][:, m0 * 128 : (m0 + 1) * 128],
                    in_=w_inT_d[i * 128 : (i + 1) * 128, m0 * 128 : (m0 + 1) * 128],
                )
            for i in range(H):
                nc.sync.dma_start(
                    out=xT_sb[i][:, 0:512], in_=xT_d[i * 128 : (i + 1) * 128, 0:512]
                )
            nc.sync.dma_start(out=gvec_sb, in_=gvec_d[:, :])
            nc.sync.dma_start(out=vd_sb, in_=vdiag_d[:, :])
            nc.sync.dma_start(out=gd_sb, in_=gdiag_d[:, :])
            for m in range(H):
                nc.sync.dma_start(
                    out=b_in_sb[:, m : m + 1], in_=b_in_d[m * 128 : (m + 1) * 128, :]
                )
            for i in range(H):
                nc.sync.dma_start(
                    out=xT_sb[i][:, 512:TH], in_=xT_d[i * 128 : (i + 1) * 128, 512:TH]
                )

            with tc.tile_pool(name="ps1", bufs=2, space="PSUM") as pp1, tc.tile_pool(
                name="ps3", bufs=3, space="PSUM"
            ) as pp3, tc.tile_pool(
                name="pso", bufs=2, space="PSUM"
            ) as pp_o, tc.tile_pool(name="pband", bufs=2) as p_band, tc.tile_pool(
                name="pcs", bufs=2
            ) as p_cs, tc.tile_pool(name="pxtd", bufs=3) as p_xtd, tc.tile_pool(
                name="pchain", bufs=2
            ) as p_ch:
                band_tiles = {}
                cs_tiles = {}
                xtd_tiles = {}
                chain_tiles = {}

                def issue_w_cols(m):
                    for i in range(H):
                        nc.sync.dma_start(
                            out=w_sb[i][:, m * 128 : (m + 1) * 128],
                            in_=w_inT_d[
                                i * 128 : (i + 1) * 128, m * 128 : (m + 1) * 128
                            ],
                        )

                def issue_band_dma(h):
                    boff = _band_off(h)
                    bw = NT * _tile_cols(h)
                    bt = p_band.tile([128, bw], bf16, name=f"band{h}", tag="band")
                    half = (bw // 2) // 2 * 2
                    nc.sync.dma_start(
                        out=bt[:, 0:half], in_=band_d[:, boff : boff + half]
                    )
                    nc.sync.dma_start(
                        out=bt[:, half:bw], in_=band_d[:, boff + half : boff + bw]
                    )
                    band_tiles[h] = bt

                def s1_group(m):
                    for c0, cn in S1CH:
                        ps1 = pp1.tile([128, 512], f32, name="ps1", tag="ps1")
                        for i in range(H):
                            nc.tensor.matmul(
                                ps1[:, :cn],
                                w_sb[i][:, m * 128 : (m + 1) * 128],
                                xT_sb[i][:, c0 : c0 + cn],
                                start=(i == 0),
                                stop=(i == H - 1),
                            )
                        nc.scalar.activation(
                            out=xp_sb[m][:, LP - PAD + c0 : LP - PAD + c0 + cn],
                            in_=ps1[:, :cn],
                            func=IDENT,
                            bias=b_in_sb[:, m : m + 1],
                            scale=1.0,
                        )
                    xtd = p_xtd.tile([128, XF // 128, 128], bf16, name="xtd", tag="xtd")
                    nc.sync.dma_start_transpose(out=xtd, in_=xp_sb[m])
                    xtd_tiles[m] = xtd

                def chain(eng, tile_out, taps):
                    in0, sc = taps[0]
                    eng.tensor_scalar(
                        out=tile_out, in0=in0, scalar1=sc, scalar2=None, op0=MULT
                    )
                    for in0, sc in taps[1:]:
                        eng.scalar_tensor_tensor(
                            out=tile_out,
                            in0=in0,
                            scalar=sc,
                            in1=tile_out,
                            op0=MULT,
                            op1=ADD,
                        )

                def band_stage(h):
                    k = KS[h]
                    p = k // 2
                    tcols = _tile_cols(h)
                    bt = band_tiles.pop(h)
                    xtd = xtd_tiles.pop(h)
                    cs = p_cs.tile([128, R, TPC], bf16, name=f"cs{h}", tag="cs")
                    cs_tiles[h] = cs
                    for b in range(NT):
                        o = b * tcols
                        bC = bt[:, o : o + R * 128].rearrange(
                            "q (r w) -> q r w", r=R
                        )
                        bL = bt[:, o + R * 128 : o + R * 128 + R * p].rearrange(
                            "q (r w) -> q r w", r=R
                        )
                        bR = bt[:, o + R * 128 + R * p : o + tcols].rearrange(
                            "q (r w) -> q r w", r=R
                        )
                        ps_s = pp3.tile([128, R, 128], f32, name="ps_s", tag="ps_s")
                        nc.tensor.matmul(
                            ps_s, xtd[:, b + 1, :], bC, start=True, stop=False
                        )
                        nc.tensor.matmul(
                            ps_s[:, :, 0:p], xtd[:, b, :], bL, start=False, stop=False
                        )
                        nc.tensor.matmul(
                            ps_s[:, :, 128 - p : 128],
                            xtd[:, b + 2, :],
                            bR,
                            start=False,
                            stop=True,
                        )
                        nc.scalar.copy(cs[:, :, b * 128 : (b + 1) * 128], ps_s)

                    # DVE static MAC chains (k=3,7,21 heads), overlap with
                    # next group's PE work; merged in tail_stage
                    if h not in PE_STATIC_HEADS:
                        gv = gvec_sb
                        taps = [
                            (
                                xp_sb[h][:, LP + j - p : LP + j - p + TPC],
                                gv[:, h * 21 + j : h * 21 + j + 1],
                            )
                            for j in range(k)
                        ]
                        tiles = []
                        for ci in range(0, k, 7):
                            ct = p_ch.tile(
                                [DH, TPC], bf16, name=f"ch{h}", tag=f"ch{ci // 7}"
                            )
                            chain(nc.vector, ct, taps[ci : ci + 7])
                            tiles.append(ct)
                        # merge sub-chains down to one tile (Pool takes the
                        # first combine of k=21 heads)
                        if len(tiles) == 3:
                            nc.gpsimd.tensor_add(tiles[1], tiles[1], tiles[2])
                            tiles = tiles[:2]
                        if len(tiles) == 2:
                            nc.vector.tensor_add(tiles[0], tiles[0], tiles[1])
                        chain_tiles[h] = tiles[0]

                def tail_stage(h):
                    cs = cs_tiles.pop(h)
                    on_pe = h in PE_STATIC_HEADS
                    k = KS[h]
                    p = k // 2
                    for ci, c0 in enumerate((0, 512)):
                        ps_o = pp_o.tile([128, 512], f32, name="ps_o", tag="ps_o")
                        n_mm = R + (k if on_pe else 0)
                        idx = 0
                        for r in range(R):
                            nc.tensor.matmul(
                                ps_o,
                                vd_sb[:, (h * R + r) * DH : (h * R + r + 1) * DH],
                                cs[:, r, c0 : c0 + 512],
                                start=(idx == 0),
                                stop=(idx == n_mm - 1),
                            )
                            idx += 1
                        if on_pe:
                            go = GD_OFF[h]
                            for j in range(k):
                                nc.tensor.matmul(
                                    ps_o,
                                    gd_sb[:, go + j * DH : go + (j + 1) * DH],
                                    xp_sb[h][:, LP + j - p + c0 : LP + j - p + c0 + 512],
                                    start=False,
                                    stop=(idx == n_mm - 1),
                                )
                                idx += 1
                            nc.scalar.copy(o_sb[h][:, c0 : c0 + 512], ps_o)
                        else:
                            sacc = chain_tiles[h]
                            nc.vector.tensor_add(
                                o_sb[h][:, c0 : c0 + 512],
                                ps_o,
                                sacc[:, c0 : c0 + 512],
                            )
                    if not on_pe:
                        chain_tiles.pop(h)

                # ---------------- pipelined emission ----------------
                issue_band_dma(HEADS[0])
                for gi, m in enumerate(HEADS):
                    if gi + 1 < H:
                        issue_w_cols(HEADS[gi + 1])
                        issue_band_dma(HEADS[gi + 1])
                    if gi == 1:
                        for i in range(H):
                            nc.sync.dma_start(
                                out=wo_sb[i],
                                in_=w_outT_d[i * 128 : (i + 1) * 128, :],
                            )
                    s1_group(m)
                    if gi >= 1:
                        band_stage(HEADS[gi - 1])
                    if gi >= 2:
                        tail_stage(HEADS[gi - 2])
                band_stage(HEADS[7])
                tail_stage(HEADS[6])
                tail_stage(HEADS[7])

            # ---------------- stage 4: out projection ----------------
            with tc.tile_pool(name="ps4", bufs=4, space="PSUM") as pp4, tc.tile_pool(
                name="post", bufs=4
            ) as p_ost:
                for t in range(NT):
                    for ei, e0 in enumerate((0, 512)):
                        ps4 = pp4.tile([128, 512], f32, name="ps4", tag="ps4")
                        n_mm = H + (1 if has_bias else 0)
                        for i in range(H):
                            nc.tensor.matmul(
                                ps4,
                                o_sb[i][:, t * 128 : (t + 1) * 128],
                                wo_sb[i][:, e0 : e0 + 512],
                                start=(i == 0),
                                stop=(i == n_mm - 1),
                            )
                        if has_bias:
                            nc.tensor.matmul(
                                ps4,
                                ones_sb,
                                bo_sb[:, e0 : e0 + 512],
                                start=False,
                                stop=True,
                            )
                        ost = p_ost.tile([128, 512], f32, name="ost", tag="ost")
                        if ei == 0:
                            nc.scalar.copy(ost, ps4)
                        else:
                            nc.vector.tensor_scalar(
                                out=ost, in0=ps4, scalar1=1.0, scalar2=None, op0=MULT
                            )
                        eng = nc.sync if ei == 0 else nc.scalar
                        eng.dma_start(
                            out=out_d[t * 128 : (t + 1) * 128, e0 : e0 + 512],
                            in_=ost,
                        )

    _split_multi_waits(nc, mybir)
    return nc


def _band_bases(A):
    """Per-head unscaled band blocks (f32): C (128,R,128), L/R (128,R,p)."""
    bases = []
    t = np.arange(128)[:, None]
    for h in range(H):
        k = KS[h]
        p = k // 2
        w = np.arange(128)[None, :]
        dC = t - w
        mC = np.abs(dC) <= p
        iC = np.clip(dC + p, 0, k - 1)
        wl = np.arange(p)[None, :] if p else np.zeros((1, 0), int)
        dL = t - wl - 128
        mL = (dL >= -p) & (dL <= p)
        iL = np.clip(dL + p, 0, k - 1)
        u = np.arange(p)[None, :] if p else np.zeros((1, 0), int)
        dR = t + p - u  # t - (128-p+u) + 128
        mR = (dR >= -p) & (dR <= p)
        iR = np.clip(dR + p, 0, k - 1)
        C = np.where(mC[:, None, :], A[h][:, iC].transpose(1, 0, 2), 0.0)
        L = np.where(mL[:, None, :], A[h][:, iL].transpose(1, 0, 2), 0.0)
        Rb = np.where(mR[:, None, :], A[h][:, iR].transpose(1, 0, 2), 0.0)
        bases.append((C, L, Rb))
    return bases


def _host_prep(inputs):
    x = np.ascontiguousarray(np.asarray(inputs["x"], dtype=np.float32))
    W_in = np.asarray(inputs["W_in"], dtype=np.float32)
    b_in = np.asarray(inputs["b_in"], dtype=np.float32)
    W_out = np.asarray(inputs["W_out"], dtype=np.float32)
    b_out = np.asarray(inputs["b_out"], dtype=np.float32)
    Wc = np.asarray(inputs["Wc"], dtype=np.float32)
    A = np.asarray(inputs["A"], dtype=np.float32)
    V = np.asarray(inputs["V"], dtype=np.float32)
    base = np.asarray(inputs["base"], dtype=np.float32)
    alphas = np.asarray(inputs["alphas"], dtype=np.float32)

    alpha = 1.0 / (1.0 + np.exp(-alphas))
    W_inT = np.ascontiguousarray(W_in.T)
    W_outT = np.ascontiguousarray(W_out.T)
    Wc_aug = np.zeros((DM, H * R), dtype=np.float32)
    for h in range(H):
        # alpha folded into c
        Wc_aug[:, R * h : R * h + R] = alpha[h] * (
            W_inT[:, h * DH : (h + 1) * DH] @ Wc[h]
        )

    bases = _band_bases(A)

    gvec = np.zeros((DH, H, 21), dtype=np.float32)
    for h in range(H):
        k = KS[h]
        gvec[:, h, :k] = ((1.0 - alpha[h]) * base[h, :k]).T

    dd = np.arange(DH)
    vd = np.zeros((DH, H, R, DH), dtype=np.float32)
    for h in range(H):
        for r in range(R):
            vd[dd, h, r, dd] = V[h, r]
    gd = np.zeros((DH, GD_TOTAL), dtype=np.float32)
    for h in PE_STATIC_HEADS:
        k = KS[h]
        go = GD_OFF[h]
        g = (1.0 - alpha[h]) * base[h, :k]  # (k, DH)
        for j in range(k):
            gd[dd, go + j * DH + dd] = g[j]

    prep = {
        "w_inT": W_inT.astype(BF16),
        "w_outT": W_outT.astype(BF16),
        "vdiag": vd.reshape(DH, H * R * DH).astype(BF16),
        "gdiag": gd.astype(BF16),
        "gvec": gvec.reshape(DH, H * 21).copy(),
        "b_in": b_in.reshape(DM, 1).copy(),
    }
    has_bias = bool(np.any(b_out != 0.0))
    if has_bias:
        prep["b_out"] = b_out.reshape(1, DM).astype(BF16)

    xT_slices = []
    band_slices = []
    per_b = NC // B
    for c in range(NC):
        bb = c // per_b
        s = (c % per_b) * TPC
        sl = np.zeros((TH, DM), dtype=np.float32)
        lo, hi = s - PAD, s + TPC + PAD
        clo, chi = max(lo, 0), min(hi, N)
        sl[clo - lo : chi - lo] = x[bb, clo:chi]
        xT_slices.append(np.ascontiguousarray(sl.T).astype(BF16))
        cc = (sl[PAD : PAD + TPC] @ Wc_aug).T.reshape(H, R, TPC)  # alpha*c

        band = np.empty((128, BAND_TOTAL), dtype=np.float32)
        for h in range(H):
            k = KS[h]
            p = k // 2
            C, L, Rb = bases[h]
            tcols = _tile_cols(h)
            boff = _band_off(h)
            ch = cc[h]  # (R, TPC)
            for b in range(NT):
                o = boff + b * tcols
                cw = ch[None, :, b * 128 : (b + 1) * 128]  # (1, R, 128)
                band[:, o : o + R * 128] = (C * cw).reshape(128, R * 128)
                if p:
                    cl = ch[None, :, b * 128 : b * 128 + p]
                    band[:, o + R * 128 : o + R * 128 + R * p] = (L * cl).reshape(
                        128, R * p
                    )
                    cr = ch[None, :, (b + 1) * 128 - p : (b + 1) * 128]
                    band[:, o + R * 128 + R * p : o + tcols] = (Rb * cr).reshape(
                        128, R * p
                    )
        band_slices.append(band.astype(BF16))
    return prep, xT_slices, band_slices, has_bias


def _run(inputs, trace=False, **kwargs):
    _install_ntff_hook_shim()
    from concourse.bass_utils import run_bass_kernel_spmd

    prep, xT_slices, band_slices, has_bias = _host_prep(inputs)
    key = ("mod", has_bias)
    if key not in _MODULE_CACHE:
        _MODULE_CACHE[key] = _build_module(has_bias)
    nc = _MODULE_CACHE[key]

    in_maps = []
    for c in range(NC):
        m = dict(prep)
        m["xT"] = xT_slices[c]
        m["band"] = band_slices[c]
        in_maps.append(m)

    res = run_bass_kernel_spmd(
        nc, in_maps, core_ids=list(range(NC)), trace=trace, **kwargs
    )
    outs = [res.results[c]["out"] for c in range(NC)]
    full = np.concatenate(outs, axis=0).reshape(B, N, DM).astype(np.float32)
    return full, res


def kernel(**inputs) -> np.ndarray:
    return _run(inputs)[0]


# revision 11
# speedup vs baseline: 1.8036x; 1.1905x over previous
"""Trainium2 Bass kernel for nn_DKAModule (dynamic-kernel attention), v3.

Data-parallel over B*n = 8192 tokens -> TPC=1024 per core (+10-token halo).
All matmuls bf16 (1 cycle/col on PE at 2.4GHz when back-to-back).

Per core, software-pipelined over heads (group g runs stage1 of head g,
band matmuls of head g-1, diag/chain tail of head g-2):

  stage1:  xp_m = W_in-block^T @ x^T + b_in     (PE; Act evac to bf16)
  band:    xtd  = 10-tile DMA transpose of xp_h (1 trigger, SP)
           ps_s = per-128-token-tile banded-conv matmuls (PE: C + L/R
                  halo slices, accumulated in PSUM). The per-token
                  coefficients c (= alpha * x_proj_h @ Wc) are folded
                  into the band matrices ON HOST, so ps_s = S*c already.
           cs   = plain PSUM evac (Act copy, bf16)
  tail:    o_h  = sum_r diag(V_r) @ cs_r        (PE, vdiag)
                + static conv:  k=11 heads via diag matmuls (PE, gdiag)
                                k=3,7,21 heads via DVE STT MAC chains
                  (chains <= 7 long for bf16 accumulation error)
  stage4:  out  = o^T-blocks @ W_out^T (+ b_out) (PE; Act/DVE evac)
"""
import sys
import types

import ml_dtypes
import numpy as np

BF16 = ml_dtypes.bfloat16

KS = [3, 3, 7, 7, 11, 11, 21, 21]
H, DM, DH, R, B, N = 8, 1024, 128, 4, 2, 4096
NC = 8
TPC = B * N // NC
PAD = 10
LP = 128  # left zero-pad columns in xp
XF = 1280  # padded xp width = 10 transpose tiles
NT = TPC // 128  # 8 token tiles
TH = TPC + 2 * PAD  # 1044 valid x columns
HEADS = (6, 7, 4, 5, 2, 3, 1, 0)  # k=21 first (long DVE chains), k=3 last
PE_STATIC_HEADS = (4, 5)  # k=11 static conv via PE diag matmuls
S1CH = [(0, 512), (512, 512), (1024, 20)]

_MODULE_CACHE = {}


def _install_ntff_hook_shim():
    """This image's antenv lacks axon_hooks; provide it so profiling works."""
    if "antenv.axon_hooks" in sys.modules:
        return
    try:
        from trn_agent_boot.trn_boot import _ntff_profile_via_ctypes

        hook = _ntff_profile_via_ctypes("/opt/axon/libaxon_pjrt.so")
    except Exception:
        hook = None
    mod = types.ModuleType("antenv.axon_hooks")
    mod.get_axon_ntff_profile_hook = lambda: hook
    mod.set_axon_ntff_profile_hook = lambda h: None
    sys.modules["antenv.axon_hooks"] = mod


def _split_multi_waits(nc, mybir):
    """walrus codegen allows a single sync-wait per instruction; hoist
    extras onto a chain of single-wait NoOps on the same engine."""
    for f in nc.m.functions:
        for blk in f.blocks:
            new_insts = []
            for inst in blk.instructions:
                si = getattr(inst, "sync_info", None)
                ow = list(si.on_wait) if si and si.on_wait else []
                if len(ow) >= 2:
                    for i, w in enumerate(ow[:-1]):
                        new_insts.append(
                            mybir.InstNoOp(
                                name=f"{inst.name}-wn{i}",
                                ins=[],
                                outs=[],
                                engine=inst.engine,
                                sync_info=mybir.SyncInfo(on_wait=[w], on_update=[]),
                            )
                        )
                    inst.sync_info = mybir.SyncInfo(
                        on_wait=[ow[-1]],
                        on_update=list(si.on_update) if si.on_update else [],
                    )
                new_insts.append(inst)
            blk.instructions = new_insts


def _tile_cols(h):
    """Band cols per token-tile for head h: C (R*128) + L (R*p) + R (R*p)."""
    p = KS[h] // 2
    return R * (128 + 2 * p)


def _band_off(h):
    """Column offset of head h's packed per-tile band blocks."""
    off = 0
    for g in range(h):
        off += NT * _tile_cols(g)
    return off


BAND_TOTAL = _band_off(H - 1) + NT * _tile_cols(H - 1)

# gdiag packing (PE-static heads only)
GD_OFF = {}
_o = 0
for _h in PE_STATIC_HEADS:
    GD_OFF[_h] = _o
    _o += KS[_h] * DH
GD_TOTAL = _o


def _build_module(has_bias):
    import concourse.bass as bass
    import concourse.tile as tile
    from concourse import mybir

    f32 = mybir.dt.float32
    bf16 = mybir.dt.bfloat16
    MULT = mybir.AluOpType.mult
    ADD = mybir.AluOpType.add
    IDENT = mybir.ActivationFunctionType.Identity

    nc = bass.Bass(trn_type="TRN2")

    xT_d = nc.dram_tensor("xT", [DM, TH], bf16, kind="ExternalInput")
    w_inT_d = nc.dram_tensor("w_inT", [DM, DM], bf16, kind="ExternalInput")
    w_outT_d = nc.dram_tensor("w_outT", [DM, DM], bf16, kind="ExternalInput")
    band_d = nc.dram_tensor("band", [128, BAND_TOTAL], bf16, kind="ExternalInput")
    vdiag_d = nc.dram_tensor("vdiag", [DH, H * R * DH], bf16, kind="ExternalInput")
    gdiag_d = nc.dram_tensor("gdiag", [DH, GD_TOTAL], bf16, kind="ExternalInput")
    gvec_d = nc.dram_tensor("gvec", [DH, H * 21], f32, kind="ExternalInput")
    b_in_d = nc.dram_tensor("b_in", [128, H], f32, kind="ExternalInput")
    if has_bias:
        b_out_d = nc.dram_tensor("b_out", [1, DM], bf16, kind="ExternalInput")
    out_d = nc.dram_tensor("out", [TPC, DM], f32, kind="ExternalOutput")

    with tile.TileContext(nc) as tc:
        with tc.tile_pool(name="const", bufs=1) as pc:
            xp_sb = [pc.tile([DH, XF], bf16, name=f"xp{m}") for m in range(H)]
            o_sb = [pc.tile([DH, TPC], bf16, name=f"o{h}") for h in range(H)]
            w_sb = [pc.tile([128, DM], bf16, name=f"w_in{i}") for i in range(H)]
            xT_sb = [pc.tile([128, TH], bf16, name=f"xT{i}") for i in range(H)]
            wo_sb = [pc.tile([128, DM], bf16, name=f"w_out{i}") for i in range(H)]
            gvec_sb = pc.tile([DH, H * 21], f32, name="gvec_sb")
            vd_sb = pc.tile([DH, H * R * DH], bf16, name="vd_sb")
            gd_sb = pc.tile([DH, GD_TOTAL], bf16, name="gd_sb")
            b_in_sb = pc.tile([128, H], f32, name="b_in_sb")
            if has_bias:
                ones_sb = pc.tile([1, 128], bf16, name="ones_sb")
                bo_sb = pc.tile([1, DM], bf16, name="bo_sb")
                nc.gpsimd.memset(ones_sb, 1.0)
                nc.sync.dma_start(out=bo_sb, in_=b_out_d[:, :])

            for m in range(H):
                nc.gpsimd.memset(xp_sb[m][:, 0 : LP - PAD], 0)
                nc.gpsimd.memset(xp_sb[m][:, LP + TPC + PAD : XF], 0)

            # warm the activation table off the critical path
            warm = pc.tile([1, 2], f32, name="warm")
            nc.gpsimd.memset(warm, 0)
            nc.scalar.activation(
                out=warm[:, 1:2], in_=warm[:, 0:1], func=IDENT, bias=0.0, scale=1.0
            )

            # ---- preamble DMAs, first-needed first, spread over SP/Act ----
            m0 = HEADS[0]
            for i in range(H):
                eng = nc.sync if i % 2 == 0 else nc.scalar
                eng.dma_start(
                    out=w_sb[i][:, m0 * 128 : (m0 + 1) * 128],
                    in_=w_inT_d[i * 128 : (i + 1) * 128, m0 * 128 : (m0 + 1) * 128],
                )
                eng = nc.scalar if i % 2 == 0 else nc.sync
                eng.dma_start(
                    out=xT_sb[i][:, 0:512], in_=xT_d[i * 128 : (i + 1) * 128, 0:512]
                )
            nc.sync.dma_start(out=b_in_sb, in_=b_in_d[:, :])
            nc.sync.dma_start(out=gvec_sb, in_=gvec_d[:, :])
            nc.sync.dma_start(out=vd_sb, in_=vdiag_d[:, :])
            nc.sync.dma_start(out=gd_sb, in_=gdiag_d[:, :])
            for i in range(H):
                eng = nc.scalar if i % 2 == 0 else nc.sync
                eng.dma_start(
                    out=xT_sb[i][:, 512:TH], in_=xT_d[i * 128 : (i + 1) * 128, 512:TH]
                )
            # rest of w_in as full row-blocks, skipping the m0 columns
            for i in range(H):
                eng = nc.sync if i % 2 == 0 else nc.scalar
                eng.dma_start(
                    out=w_sb[i][:, 0 : m0 * 128],
                    in_=w_inT_d[i * 128 : (i + 1) * 128, 0 : m0 * 128],
                )
                eng.dma_start(
                    out=w_sb[i][:, (m0 + 1) * 128 : DM],
                    in_=w_inT_d[i * 128 : (i + 1) * 128, (m0 + 1) * 128 : DM],
                )

            with tc.tile_pool(name="ps1", bufs=2, space="PSUM") as pp1, tc.tile_pool(
                name="ps3", bufs=3, space="PSUM"
            ) as pp3, tc.tile_pool(
                name="pso", bufs=3, space="PSUM"
            ) as pp_o, tc.tile_pool(name="pband", bufs=3) as p_band, tc.tile_pool(
                name="pcs", bufs=2
            ) as p_cs, tc.tile_pool(name="pxtd", bufs=3) as p_xtd, tc.tile_pool(
                name="pchain", bufs=2
            ) as p_ch:
                band_tiles = {}
                cs_tiles = {}
                xtd_tiles = {}
                chain_tiles = {}

                def issue_band_dma(h):
                    boff = _band_off(h)
                    bw = NT * _tile_cols(h)
                    bt = p_band.tile([128, bw], bf16, name=f"band{h}", tag="band")
                    nc.sync.dma_start(out=bt, in_=band_d[:, boff : boff + bw])
                    band_tiles[h] = bt

                def s1_group(m):
                    for c0, cn in S1CH:
                        ps1 = pp1.tile([128, 512], f32, name="ps1", tag="ps1")
                        for i in range(H):
                            nc.tensor.matmul(
                                ps1[:, :cn],
                                w_sb[i][:, m * 128 : (m + 1) * 128],
                                xT_sb[i][:, c0 : c0 + cn],
                                start=(i == 0),
                                stop=(i == H - 1),
                            )
                        nc.scalar.activation(
                            out=xp_sb[m][:, LP - PAD + c0 : LP - PAD + c0 + cn],
                            in_=ps1[:, :cn],
                            func=IDENT,
                            bias=b_in_sb[:, m : m + 1],
                            scale=1.0,
                        )
                    xtd = p_xtd.tile([128, XF // 128, 128], bf16, name="xtd", tag="xtd")
                    nc.sync.dma_start_transpose(out=xtd, in_=xp_sb[m])
                    xtd_tiles[m] = xtd

                def chain(eng, tile_out, taps):
                    in0, sc = taps[0]
                    eng.tensor_scalar(
                        out=tile_out, in0=in0, scalar1=sc, scalar2=None, op0=MULT
                    )
                    for in0, sc in taps[1:]:
                        eng.scalar_tensor_tensor(
                            out=tile_out,
                            in0=in0,
                            scalar=sc,
                            in1=tile_out,
                            op0=MULT,
                            op1=ADD,
                        )

                def band_stage(h):
                    k = KS[h]
                    p = k // 2
                    tcols = _tile_cols(h)
                    bt = band_tiles.pop(h)
                    xtd = xtd_tiles.pop(h)
                    cs = p_cs.tile([128, R, TPC], bf16, name=f"cs{h}", tag="cs")
                    cs_tiles[h] = cs
                    for b in range(NT):
                        o = b * tcols
                        bC = bt[:, o : o + R * 128].rearrange(
                            "q (r w) -> q r w", r=R
                        )
                        bL = bt[:, o + R * 128 : o + R * 128 + R * p].rearrange(
                            "q (r w) -> q r w", r=R
                        )
                        bR = bt[:, o + R * 128 + R * p : o + tcols].rearrange(
                            "q (r w) -> q r w", r=R
                        )
                        ps_s = pp3.tile([128, R, 128], f32, name="ps_s", tag="ps_s")
                        nc.tensor.matmul(
                            ps_s, xtd[:, b + 1, :], bC, start=True, stop=False
                        )
                        nc.tensor.matmul(
                            ps_s[:, :, 0:p], xtd[:, b, :], bL, start=False, stop=False
                        )
                        nc.tensor.matmul(
                            ps_s[:, :, 128 - p : 128],
                            xtd[:, b + 2, :],
                            bR,
                            start=False,
                            stop=True,
                        )
                        nc.scalar.copy(cs[:, :, b * 128 : (b + 1) * 128], ps_s)

                    # DVE static MAC chains (k=3,7,21 heads), overlap with
                    # next group's PE work; merged in tail_stage
                    if h not in PE_STATIC_HEADS:
                        gv = gvec_sb
                        taps = [
                            (
                                xp_sb[h][:, LP + j - p : LP + j - p + TPC],
                                gv[:, h * 21 + j : h * 21 + j + 1],
                            )
                            for j in range(k)
                        ]
                        tiles = []
                        for ci in range(0, k, 7):
                            ct = p_ch.tile(
                                [DH, TPC], bf16, name=f"ch{h}", tag=f"ch{ci // 7}"
                            )
                            chain(nc.vector, ct, taps[ci : ci + 7])
                            tiles.append(ct)
                        # merge sub-chains down to one tile (Pool takes the
                        # first combine of k=21 heads)
                        if len(tiles) == 3:
                            nc.gpsimd.tensor_add(tiles[1], tiles[1], tiles[2])
                            tiles = tiles[:2]
                        if len(tiles) == 2:
                            nc.vector.tensor_add(tiles[0], tiles[0], tiles[1])
                        chain_tiles[h] = tiles[0]

                def tail_stage(h):
                    cs = cs_tiles.pop(h)
                    on_pe = h in PE_STATIC_HEADS
                    k = KS[h]
                    p = k // 2
                    for ci, c0 in enumerate((0, 512)):
                        ps_o = pp_o.tile([128, 512], f32, name="ps_o", tag="ps_o")
                        n_mm = R + (k if on_pe else 0)
                        idx = 0
                        for r in range(R):
                            nc.tensor.matmul(
                                ps_o,
                                vd_sb[:, (h * R + r) * DH : (h * R + r + 1) * DH],
                                cs[:, r, c0 : c0 + 512],
                                start=(idx == 0),
                                stop=(idx == n_mm - 1),
                            )
                            idx += 1
                        if on_pe:
                            go = GD_OFF[h]
                            for j in range(k):
                                nc.tensor.matmul(
                                    ps_o,
                                    gd_sb[:, go + j * DH : go + (j + 1) * DH],
                                    xp_sb[h][:, LP + j - p + c0 : LP + j - p + c0 + 512],
                                    start=False,
                                    stop=(idx == n_mm - 1),
                                )
                                idx += 1
                            nc.scalar.copy(o_sb[h][:, c0 : c0 + 512], ps_o)
                        else:
                            sacc = chain_tiles[h]
                            nc.vector.tensor_add(
                                o_sb[h][:, c0 : c0 + 512],
                                ps_o,
                                sacc[:, c0 : c0 + 512],
                            )
                    if not on_pe:
                        chain_tiles.pop(h)

                # ---------------- pipelined emission ----------------
                issue_band_dma(HEADS[0])
                issue_band_dma(HEADS[1])
                for gi, m in enumerate(HEADS):
                    if gi + 2 < H:
                        issue_band_dma(HEADS[gi + 2])
                    if gi == 3:
                        for i in range(H):
                            nc.sync.dma_start(
                                out=wo_sb[i],
                                in_=w_outT_d[i * 128 : (i + 1) * 128, :],
                            )
                    s1_group(m)
                    if gi >= 1:
                        band_stage(HEADS[gi - 1])
                    if gi >= 2:
                        tail_stage(HEADS[gi - 2])
                band_stage(HEADS[7])
                tail_stage(HEADS[6])
                tail_stage(HEADS[7])

            # ---------------- stage 4: out projection ----------------
            with tc.tile_pool(name="ps4", bufs=4, space="PSUM") as pp4, tc.tile_pool(
                name="post", bufs=4
            ) as p_ost:
                for t in range(NT):
                    for ei, e0 in enumerate((0, 512)):
                        ps4 = pp4.tile([128, 512], f32, name="ps4", tag="ps4")
                        n_mm = H + (1 if has_bias else 0)
                        for i in range(H):
                            nc.tensor.matmul(
                                ps4,
                                o_sb[i][:, t * 128 : (t + 1) * 128],
                                wo_sb[i][:, e0 : e0 + 512],
                                start=(i == 0),
                                stop=(i == n_mm - 1),
                            )
                        if has_bias:
                            nc.tensor.matmul(
                                ps4,
                                ones_sb,
                                bo_sb[:, e0 : e0 + 512],
                                start=False,
                                stop=True,
                            )
                        ost = p_ost.tile([128, 512], f32, name="ost", tag="ost")
                        nc.vector.tensor_scalar(
                            out=ost, in0=ps4, scalar1=1.0, scalar2=None, op0=MULT
                        )
                        eng = nc.sync if ei == 0 else nc.scalar
                        eng.dma_start(
                            out=out_d[t * 128 : (t + 1) * 128, e0 : e0 + 512],
                            in_=ost,
                        )

    _split_multi_waits(nc, mybir)
    return nc


def _band_bases(A):
    """Per-head unscaled band blocks (f32): C (128,R,128), L/R (128,R,p)."""
    bases = []
    t = np.arange(128)[:, None]
    for h in range(H):
        k = KS[h]
        p = k // 2
        w = np.arange(128)[None, :]
        dC = t - w
        mC = np.abs(dC) <= p
        iC = np.clip(dC + p, 0, k - 1)
        wl = np.arange(p)[None, :] if p else np.zeros((1, 0), int)
        dL = t - wl - 128
        mL = (dL >= -p) & (dL <= p)
        iL = np.clip(dL + p, 0, k - 1)
        u = np.arange(p)[None, :] if p else np.zeros((1, 0), int)
        dR = t + p - u  # t - (128-p+u) + 128
        mR = (dR >= -p) & (dR <= p)
        iR = np.clip(dR + p, 0, k - 1)
        C = np.where(mC[:, None, :], A[h][:, iC].transpose(1, 0, 2), 0.0)
        L = np.where(mL[:, None, :], A[h][:, iL].transpose(1, 0, 2), 0.0)
        Rb = np.where(mR[:, None, :], A[h][:, iR].transpose(1, 0, 2), 0.0)
        bases.append((C, L, Rb))
    return bases


def _host_prep(inputs):
    x = np.ascontiguousarray(np.asarray(inputs["x"], dtype=np.float32))
    W_in = np.asarray(inputs["W_in"], dtype=np.float32)
    b_in = np.asarray(inputs["b_in"], dtype=np.float32)
    W_out = np.asarray(inputs["W_out"], dtype=np.float32)
    b_out = np.asarray(inputs["b_out"], dtype=np.float32)
    Wc = np.asarray(inputs["Wc"], dtype=np.float32)
    A = np.asarray(inputs["A"], dtype=np.float32)
    V = np.asarray(inputs["V"], dtype=np.float32)
    base = np.asarray(inputs["base"], dtype=np.float32)
    alphas = np.asarray(inputs["alphas"], dtype=np.float32)

    alpha = 1.0 / (1.0 + np.exp(-alphas))
    W_inT = np.ascontiguousarray(W_in.T)
    W_outT = np.ascontiguousarray(W_out.T)
    Wc_aug = np.zeros((DM, H * R), dtype=np.float32)
    for h in range(H):
        # alpha folded into c
        Wc_aug[:, R * h : R * h + R] = alpha[h] * (
            W_inT[:, h * DH : (h + 1) * DH] @ Wc[h]
        )

    bases = _band_bases(A)

    gvec = np.zeros((DH, H, 21), dtype=np.float32)
    for h in range(H):
        k = KS[h]
        gvec[:, h, :k] = ((1.0 - alpha[h]) * base[h, :k]).T

    dd = np.arange(DH)
    vd = np.zeros((DH, H, R, DH), dtype=np.float32)
    for h in range(H):
        for r in range(R):
            vd[dd, h, r, dd] = V[h, r]
    gd = np.zeros((DH, GD_TOTAL), dtype=np.float32)
    for h in PE_STATIC_HEADS:
        k = KS[h]
        go = GD_OFF[h]
        g = (1.0 - alpha[h]) * base[h, :k]  # (k, DH)
        for j in range(k):
            gd[dd, go + j * DH + dd] = g[j]

    prep = {
        "w_inT": W_inT.astype(BF16),
        "w_outT": W_outT.astype(BF16),
        "vdiag": vd.reshape(DH, H * R * DH).astype(BF16),
        "gdiag": gd.astype(BF16),
        "gvec": gvec.reshape(DH, H * 21).copy(),
        "b_in": np.ascontiguousarray(b_in.reshape(H, 128).T),
    }
    has_bias = bool(np.any(b_out != 0.0))
    if has_bias:
        prep["b_out"] = b_out.reshape(1, DM).astype(BF16)

    xT_slices = []
    band_slices = []
    per_b = NC // B
    for c in range(NC):
        bb = c // per_b
        s = (c % per_b) * TPC
        sl = np.zeros((TH, DM), dtype=np.float32)
        lo, hi = s - PAD, s + TPC + PAD
        clo, chi = max(lo, 0), min(hi, N)
        sl[clo - lo : chi - lo] = x[bb, clo:chi]
        xT_slices.append(np.ascontiguousarray(sl.T).astype(BF16))
        cc = (sl[PAD : PAD + TPC] @ Wc_aug).T.reshape(H, R, TPC)  # alpha*c

        band = np.empty((128, BAND_TOTAL), dtype=np.float32)
        for h in range(H):
            k = KS[h]
            p = k // 2
            C, L, Rb = bases[h]
            tcols = _tile_cols(h)
            boff = _band_off(h)
            ch = cc[h]  # (R, TPC)
            for b in range(NT):
                o = boff + b * tcols
                cw = ch[None, :, b * 128 : (b + 1) * 128]  # (1, R, 128)
                band[:, o : o + R * 128] = (C * cw).reshape(128, R * 128)
                if p:
                    cl = ch[None, :, b * 128 : b * 128 + p]
                    band[:, o + R * 128 : o + R * 128 + R * p] = (L * cl).reshape(
                        128, R * p
                    )
                    cr = ch[None, :, (b + 1) * 128 - p : (b + 1) * 128]
                    band[:, o + R * 128 + R * p : o + tcols] = (Rb * cr).reshape(
                        128, R * p
                    )
        band_slices.append(band.astype(BF16))
    return prep, xT_slices, band_slices, has_bias


def _run(inputs, trace=False, **kwargs):
    _install_ntff_hook_shim()
    from concourse.bass_utils import run_bass_kernel_spmd

    prep, xT_slices, band_slices, has_bias = _host_prep(inputs)
    key = ("mod", has_bias)
    if key not in _MODULE_CACHE:
        _MODULE_CACHE[key] = _build_module(has_bias)
    nc = _MODULE_CACHE[key]

    in_maps = []
    for c in range(NC):
        m = dict(prep)
        m["xT"] = xT_slices[c]
        m["band"] = band_slices[c]
        in_maps.append(m)

    res = run_bass_kernel_spmd(
        nc, in_maps, core_ids=list(range(NC)), trace=trace, **kwargs
    )
    outs = [res.results[c]["out"] for c in range(NC)]
    full = np.concatenate(outs, axis=0).reshape(B, N, DM).astype(np.float32)
    return full, res


def kernel(**inputs) -> np.ndarray:
    return _run(inputs)[0]
